# revision 2
# baseline (speedup 1.0000x reference)
"""Megatron-style tensor-parallel causal attention (BitLinear qkv/o) on 8 TRN2 cores.

v3: all-bf16 datapath, software-pipelined emission for dense PE.
- QKV proj per 256-token chunk; RoPE rotate-half via PE permutation matmul,
  RoPE muls on DVE, swap-matmuls deferred one step so PE never waits.
- First chunk of batch 0 runs h-group-major against a 4-way split weight
  DMA so PE starts ~3us in instead of waiting ~12us for all weights.
- Attention with exact-causal 128-granular blocks; score units emitted one
  unit ahead of the attn@v/sum matmuls so PE never waits on Exp (ACT).
  Only the exact 128x128 diagonal triangles are masked (DVE mul).
- o-proj emitted as filler jobs interleaved between attention units
  (one q-chunk behind); PSUM shared with the score-tile ring.
- Outputs bf16, partials summed across cores on host.
"""

import math
from collections import deque

import numpy as np

EPS = 1e-5
NUM_HEADS = 16
HEAD_DIM = 128
B, S, H = 2, 2048, 2048
NCORES = 8
HPC = NUM_HEADS // NCORES        # heads per core = 2
FPC = 3 * HPC * HEAD_DIM         # qkv features per core = 768
P = 128
NHT = H // P                     # 16 h_in tiles
CH = 256                         # proj token chunk
NCH = S // CH                    # 8 chunks per batch
QC = 512                         # attention q chunk
NQC = S // QC                    # 4


def _attn_units(qc):
    """Score-tile units for q-chunk qc: list of units; each unit is a list of
    (kb, psum_off, span, q_off, is_tri) covering causal k-tiles ascending."""
    units = []
    for g in range(0, 4 * qc, 2):
        units.append([(g, 0, QC, 0, False), (g + 1, QC, QC, 0, False)])
    d = 4 * qc
    units.append([(d + 0, 0, 512, 0, True), (d + 1, 512, 384, 128, True)])
    units.append([(d + 2, 0, 256, 256, True), (d + 3, 256, 128, 384, True)])
    return units


def _L(inst, label):
    try:
        LABELS[inst.ins.name] = label
    except Exception:
        pass
    return inst


def _build_program():
    import concourse.bacc as bacc
    import concourse.mybir as mybir
    import concourse.tile as tile

    f32 = mybir.dt.float32
    bf16 = mybir.dt.bfloat16
    AF = mybir.ActivationFunctionType

    nc = bacc.Bacc(None, target_bir_lowering=False)

    xt = nc.dram_tensor("xt", [B, H, S], bf16, kind="ExternalInput")
    wf = [
        nc.dram_tensor(f"wf{f}", [P, NHT * P], bf16, kind="ExternalInput")
        for f in range(4)
    ]
    wv = nc.dram_tensor("wv", [P, NHT * 2 * P], bf16, kind="ExternalInput")
    wo = nc.dram_tensor("wo", [P, HPC * H], bf16, kind="ExternalInput")
    cos_t = nc.dram_tensor("cos_t", [P, S], bf16, kind="ExternalInput")
    sin_t = nc.dram_tensor("sin_t", [P, S], bf16, kind="ExternalInput")
    pmat = nc.dram_tensor("pmat", [P, P], bf16, kind="ExternalInput")
    tri = nc.dram_tensor("tri", [P, P], bf16, kind="ExternalInput")
    ones = nc.dram_tensor("ones", [P, P], bf16, kind="ExternalInput")
    out = nc.dram_tensor("out", [B, S, H], bf16, kind="ExternalOutput")

    with tile.TileContext(nc) as tc:
        with (
            tc.tile_pool(name="const", bufs=1) as cpool,
            tc.tile_pool(name="work", bufs=1) as wpool,
            tc.psum_pool(name="ps", bufs=1) as psp,
        ):
            # weights split by f-block: wF[0..3]=(q0,q1,k0,k1), wV=(v0,v1)
            wF_sb = [cpool.tile([P, NHT * P], bf16, name=f"wF{f}") for f in range(4)]
            wV_sb = cpool.tile([P, NHT * 2 * P], bf16, name="wV")
            cos_sb = cpool.tile([P, S], bf16)
            sin_sb = cpool.tile([P, S], bf16)
            pm_sb = cpool.tile([P, P], bf16)
            wo_sb = cpool.tile([P, HPC * H], bf16)
            tri_sb = cpool.tile([P, P], bf16)
            ones_sb = cpool.tile([P, P], bf16)


            # startup-critical DMA order: first x chunk, then weights by f-block
            xt_tiles = {}

            def get_xt(b, tcn):
                key = (b, tcn)
                if key not in xt_tiles:
                    t = wpool.tile(
                        [P, NHT, CH], bf16, tag="xt", bufs=2, name=f"xt_{b}_{tcn}"
                    )
                    nc.sync.dma_start(
                        t[:],
                        xt[b, :, tcn * CH : (tcn + 1) * CH].rearrange(
                            "(t p) c -> p t c", p=P
                        ),
                    )
                    xt_tiles[key] = t
                return xt_tiles[key]

            xt0a = cpool.tile([P, NHT // 2, CH], bf16, name="xt0a")
            xt0b = cpool.tile([P, NHT // 2, CH], bf16, name="xt0b")
            nc.sync.dma_start(wF_sb[0][:], wf[0][:])
            nc.sync.dma_start(
                xt0a[:], xt[0, 0 : H // 2, 0:CH].rearrange("(t p) c -> p t c", p=P)
            )
            nc.sync.dma_start(
                xt0b[:], xt[0, H // 2 : H, 0:CH].rearrange("(t p) c -> p t c", p=P)
            )
            nc.sync.dma_start(wF_sb[1][:], wf[1][:])
            nc.sync.dma_start(pm_sb[:], pmat[:])
            nc.sync.dma_start(cos_sb[:], cos_t[:])
            nc.sync.dma_start(sin_sb[:], sin_t[:])
            for f in range(2, 4):
                nc.sync.dma_start(wF_sb[f][:], wf[f][:])
            nc.sync.dma_start(wV_sb[:], wv[:])
            nc.sync.dma_start(wo_sb[:], wo[:])
            nc.sync.dma_start(tri_sb[:], tri[:])
            nc.sync.dma_start(ones_sb[:], ones[:])

            # persistent per-batch q/k (roped), v; y per (b, hl, qc)
            qk = [
                [cpool.tile([P, S], bf16, name=f"qk{f}_{b}") for f in range(4)]
                for b in range(B)
            ]
            v_sb = [cpool.tile([P, S * HPC], bf16, name=f"v_{b}") for b in range(B)]
            y_t = {
                (b, hl, qc): cpool.tile([P, QC], bf16, name=f"y_{b}_{hl}_{qc}")
                for b in range(B)
                for hl in range(HPC)
                for qc in range(NQC)
            }

            pending = deque()  # deferred RoPE flushes: (b, pair, raw, slice)

            def flush_one():
                if not pending:
                    return
                fb, pr, praw, pcs = pending.popleft()
                ps_sw = psp.tile([P, 2 * CH], f32, tag="sum", bufs=1)
                _L(nc.tensor.matmul(
                    ps_sw[:], lhsT=pm_sb[:], rhs=praw[:], start=True, stop=True
                ), f"proj-sw b{fb} pr{pr}")
                for half in range(2):
                    pf = 2 * pr + half
                    hs = slice(half * CH, (half + 1) * CH)
                    m2 = wpool.tile([P, CH], bf16, tag="m2", bufs=2)
                    nc.vector.tensor_mul(m2[:], ps_sw[:, hs], sin_sb[:, pcs])
                    m1 = wpool.tile([P, CH], bf16, tag="m1", bufs=2)
                    nc.vector.tensor_mul(m1[:], praw[:, hs], cos_sb[:, pcs])
                    nc.vector.tensor_add(qk[fb][pf][:, pcs], m1[:], m2[:])

            def proj(b):
                for tcn in range(NCH):
                    cs = slice(tcn * CH, (tcn + 1) * CH)
                    if (b, tcn) == (0, 0):
                        xth = lambda h: (xt0a if h < 8 else xt0b)[:, h % 8, :]
                    else:
                        xt_sb = get_xt(b, tcn)
                        xth = lambda h: xt_sb[:, h, :]
                    for pr in range(2):
                        ps = psp.tile([P, 2 * QC], f32, tag="sc", bufs=2)
                        for half in range(2):
                            for h in range(NHT):
                                _L(nc.tensor.matmul(
                                    ps[:, half * CH : (half + 1) * CH],
                                    lhsT=wF_sb[2 * pr + half][:, h * P : (h + 1) * P],
                                    rhs=xth(h),
                                    start=(h == 0),
                                    stop=(h == NHT - 1),
                                ), f"proj-qk b{b} c{tcn} pr{pr} h{h}")
                        raw = wpool.tile([P, 2 * CH], bf16, tag="raw", bufs=3)
                        nc.scalar.copy(raw[:], ps[:, 0 : 2 * CH])
                        pending.append((b, pr, raw, cs))
                        if len(pending) >= 2:
                            flush_one()
                        drain_jobs(1)
                    psv = psp.tile([P, QC], f32, tag="yt", bufs=1)
                    for tsub in range(2):
                        for h in range(NHT):
                            _L(nc.tensor.matmul(
                                psv[:, tsub * 2 * P : (tsub + 1) * 2 * P],
                                lhsT=xth(h)[:, tsub * P : (tsub + 1) * P],
                                rhs=wV_sb[:, h * 2 * P : (h + 1) * 2 * P],
                                start=(h == 0),
                                stop=(h == NHT - 1),
                            ), f"proj-v b{b} c{tcn} t{tsub} h{h}")
                    nc.scalar.copy(
                        v_sb[b][:, tcn * 4 * P : (tcn + 1) * 4 * P], psv[:]
                    )
                    flush_one()
                    drain_jobs(1)
                while pending:
                    flush_one()

            # o-proj filler jobs: mm phase and copy phase staggered by one
            jobs = deque()
            cps = deque()
            oscnt = [0]

            def push_oproj(b, qc):
                for tt in range(4):
                    for oc in range(4):
                        jobs.append((b, qc, tt, oc))

            def job_mms(b, qc, tt, oc):
                ops = psp.tile([P, QC], f32, tag="op", bufs=2)
                for hl in range(HPC):
                    _L(nc.tensor.matmul(
                        ops[:],
                        lhsT=y_t[(b, hl, qc)][:, tt * P : (tt + 1) * P],
                        rhs=wo_sb[:, hl * H + oc * QC : hl * H + (oc + 1) * QC],
                        start=(hl == 0),
                        stop=(hl == HPC - 1),
                    ), f"oproj b{b} q{qc} t{tt} o{oc} h{hl}")
                cps.append((ops, b, qc, tt, oc))

            def job_copy():
                ops, b, qc, tt, oc = cps.popleft()
                os_sb = wpool.tile([P, QC], bf16, tag="os", bufs=4)
                h0 = (oscnt[0] % 2) * QC // 2
                h1 = QC // 2 - h0
                nc.vector.tensor_copy(
                    os_sb[:, h0 : h0 + QC // 2], ops[:, h0 : h0 + QC // 2]
                )
                nc.scalar.copy(
                    os_sb[:, h1 : h1 + QC // 2], ops[:, h1 : h1 + QC // 2]
                )
                oscnt[0] += 1
                nc.sync.dma_start(
                    out[
                        b,
                        qc * QC + tt * P : qc * QC + (tt + 1) * P,
                        oc * QC : (oc + 1) * QC,
                    ],
                    os_sb[:],
                )

            def drain_jobs(k):
                for _ in range(k):
                    if jobs:
                        job_mms(*jobs.popleft())
                    if len(cps) >= 2:
                        job_copy()

            def flush_jobs():
                # final drain: fuse job pairs into [P,1024] sc-tag tiles with
                # one wide copy+DMA so the tail is PE-bound, not copy-bound
                fcps = deque()

                def fused_copy():
                    ops, b, qc, tt, oc0 = fcps.popleft()
                    os2 = wpool.tile([P, 2 * QC], bf16, tag="os2", bufs=3)
                    nc.scalar.copy(os2[:, 0:QC], ops[:, 0:QC])
                    nc.vector.tensor_copy(os2[:, QC : 2 * QC], ops[:, QC : 2 * QC])
                    nc.sync.dma_start(
                        out[
                            b,
                            qc * QC + tt * P : qc * QC + (tt + 1) * P,
                            oc0 * QC : (oc0 + 2) * QC,
                        ],
                        os2[:],
                    )

                while cps:
                    job_copy()
                while jobs:
                    b, qc, tt, oc0 = jobs.popleft()
                    _, _, _, oc1 = jobs.popleft()
                    assert oc1 == oc0 + 1
                    ops = psp.tile([P, 2 * QC], f32, tag="sc", bufs=2)
                    for j, oc in ((0, oc0), (1, oc1)):
                        for hl in range(HPC):
                            _L(nc.tensor.matmul(
                                ops[:, j * QC : (j + 1) * QC],
                                lhsT=y_t[(b, hl, qc)][:, tt * P : (tt + 1) * P],
                                rhs=wo_sb[:, hl * H + oc * QC : hl * H + (oc + 1) * QC],
                                start=(hl == 0),
                                stop=(hl == HPC - 1),
                            ), f"oprojf b{b} q{qc} t{tt} o{oc} h{hl}")
                    fcps.append((ops, b, qc, tt, oc0))
                    if len(fcps) >= 2:
                        fused_copy()
                while fcps:
                    fused_copy()

            def attn(b, qc, hl):
                units = _attn_units(qc)
                n = len(units)
                kmax = 4 * qc + 3
                yt_ps = psp.tile([P, QC], f32, tag="yt", bufs=1)
                sum_ps = psp.tile([P, QC], f32, tag="sum", bufs=1)
                ex_of = {}

                def sc_emit(u):
                    unit = units[u]
                    width = unit[-1][1] + unit[-1][2]
                    sc = psp.tile([P, 2 * QC], f32, tag="sc", bufs=2)
                    for kb, poff, span, qoff, _ in unit:
                        _L(nc.tensor.matmul(
                            sc[:, poff : poff + span],
                            lhsT=qk[b][2 + hl][:, kb * P : (kb + 1) * P],
                            rhs=qk[b][hl][:, qc * QC + qoff : (qc + 1) * QC],
                            start=True,
                            stop=True,
                        ), f"attn-sc b{b} q{qc} h{hl} u{u} k{kb}")
                    ex = wpool.tile([P, 2 * QC], bf16, tag="ex", bufs=3)
                    _L(nc.scalar.activation(ex[:, 0:width], sc[:, 0:width], AF.Exp), f"attn-exp b{b} q{qc} h{hl} u{u}")
                    for kb, poff, span, qoff, is_tri in unit:
                        if is_tri:
                            nc.vector.tensor_mul(
                                ex[:, poff : poff + P],
                                ex[:, poff : poff + P],
                                tri_sb[:],
                            )
                    ex_of[u] = ex

                sc_emit(0)
                for u in range(n):
                    if u + 1 < n:
                        sc_emit(u + 1)
                    ex = ex_of.pop(u)
                    for kb, poff, span, qoff, _ in units[u]:
                        _L(nc.tensor.matmul(
                            yt_ps[:, qoff:QC],
                            lhsT=v_sb[b][
                                :, kb * 2 * P + hl * P : kb * 2 * P + (hl + 1) * P
                            ],
                            rhs=ex[:, poff : poff + span],
                            start=(kb == 0),
                            stop=(kb == kmax),
                        ), f"attn-av b{b} q{qc} h{hl} u{u} k{kb}")
                        _L(nc.tensor.matmul(
                            sum_ps[:, qoff:QC],
                            lhsT=ones_sb[:],
                            rhs=ex[:, poff : poff + span],
                            start=(kb == 0),
                            stop=(kb == kmax),
                        ), f"attn-sum b{b} q{qc} h{hl} u{u} k{kb}")
                    drain_jobs(2)
                recip = wpool.tile([P, QC], f32, tag="rc", bufs=2)
                nc.vector.reciprocal(recip[:], sum_ps[:])
                nc.vector.tensor_mul(y_t[(b, hl, qc)][:], yt_ps[:], recip[:])

            for b in range(B):
                proj(b)
                if b + 1 < B:  # prefetch next batch's first chunks
                    get_xt(b + 1, 0)
                    get_xt(b + 1, 1)
                for qc in range(NQC - 1, -1, -1):
                    attn(b, qc, 0)
                    attn(b, qc, 1)
                    push_oproj(b, qc)
            flush_jobs()
    nc.finalize()
    return nc


_NC_CACHE = None
LABELS = {}


def _get_program():
    global _NC_CACHE
    if _NC_CACHE is None:
        _NC_CACHE = _build_program()
    return _NC_CACHE


def kernel(x, rotary, qkv_weight, o_weight):
    import jax
    import jax.numpy as jnp
    import ml_dtypes
    from concourse.bass_utils import run_bass_kernel_spmd

    bf = ml_dtypes.bfloat16

    cpu = jax.devices("cpu")[0]
    with jax.default_device(cpu):
        sq = jnp.mean(jnp.abs(jnp.asarray(qkv_weight)))
        wq_q = np.asarray(jnp.round(jnp.asarray(qkv_weight) / (sq + EPS)), np.float32)
        so = jnp.mean(jnp.abs(jnp.asarray(o_weight)))
        wo_q = np.asarray(jnp.round(jnp.asarray(o_weight) / (so + EPS)), np.float32)
        sq = float(sq)
        so = float(so)

    xt = np.ascontiguousarray(np.asarray(x).transpose(0, 2, 1)).astype(bf)
    cos_t = np.ascontiguousarray(np.asarray(rotary)[1].T).astype(bf)
    sin_t = np.ascontiguousarray(np.asarray(rotary)[0].T).astype(bf)

    # rotate-half permutation (with sign) as a matmul lhsT:
    # out[m] = -q[m+64] for m<64 ; +q[m-64] for m>=64
    pmat = np.zeros((P, P), np.float32)
    for m in range(64):
        pmat[m + 64, m] = -1.0
        pmat[m, m + 64] = 1.0
    pmat = pmat.astype(bf)

    kk = np.arange(P)[:, None]
    qq = np.arange(P)[None, :]
    tri = (qq >= kk).astype(np.float32).astype(bf)  # [k, q] keep q>=k
    ones = np.ones((P, P), np.float32).astype(bf)

    sm_scale = np.float32(sq * sq / math.sqrt(HEAD_DIM))
    final_scale = sq * so

    in_maps = []
    for c in range(NCORES):
        rows = []
        for part in range(3):  # q, k, v blocks of qkv_weight
            for hl in range(HPC):
                g = HPC * c + hl
                blk = wq_q[part * H + g * HEAD_DIM : part * H + (g + 1) * HEAD_DIM]
                if part == 0:
                    blk = blk * sm_scale
                rows.append(blk)
        wqkv_c = np.concatenate(rows, axis=0).T  # [H, 768]
        # rearrange rows (t p) -> [p, t*...] so DMA rows are contiguous
        wr = wqkv_c.reshape(NHT, P, FPC).transpose(1, 0, 2)  # [p, t, 768]
        wf_c = {
            f"wf{f}": np.ascontiguousarray(
                wr[:, :, f * P : (f + 1) * P].reshape(P, NHT * P)
            ).astype(bf)
            for f in range(4)
        }
        wv_c = np.ascontiguousarray(
            wr[:, :, 4 * P : 6 * P].reshape(P, NHT * 2 * P)
        ).astype(bf)
        wo_full = wo_q[:, c * FPC // 3 : (c + 1) * FPC // 3].T * final_scale  # [256, H]
        wo_c = np.ascontiguousarray(
            wo_full.reshape(HPC, P, H).transpose(1, 0, 2).reshape(P, HPC * H)
        ).astype(bf)
        in_maps.append(
            {
                "xt": xt,
                **wf_c,
                "wv": wv_c,
                "wo": wo_c,
                "cos_t": cos_t,
                "sin_t": sin_t,
                "pmat": pmat,
                "tri": tri,
                "ones": ones,
            }
        )

    nc = _get_program()
    global _last_in_maps
    _last_in_maps = in_maps
    res = run_bass_kernel_spmd(nc, in_maps, core_ids=list(range(NCORES)))
    acc = res.results[0]["out"].astype(np.float32)
    for c in range(1, NCORES):
        acc = acc + res.results[c]["out"].astype(np.float32)
    return acc
